# revision 18
# baseline (speedup 1.0000x reference)
"""GQA attention (B=2, S=2048, 32 q heads / 4 kv heads, d=64, hidden=2048)
distributed over 8 TRN2 NeuronCores.

Sharding: core c = (batch b = c//4, kv-group g = c%4). Each core computes the
8 query heads of its kv-group for its batch, plus the partial o_proj for its
head slice; the host sums the 4 partial outputs per batch.

Layout strategy (per core):
  - x^T tiles (hidden on partitions) produced by TensorE transposes.
  - Q^T/K^T/V^T projections straight from x^T (all matmuls in float32r:
    fp32 with 12-bit mantissa rounding, 1 cycle/row on the PE).
  - RoPE applied in "de-interleaved" head layout (host permutes Wq/Wk
    columns so pair-swap = contiguous 32-partition block swap done by DMA).
  - scores^T (t on partitions, s on free) so softmax needs no transposes:
    exp on ScalarE (no max subtraction: scores are O(10), fp32 range is
    plenty), causal masking via affine_select on diagonal tiles only,
    row-sums come free as an extra ones-column in the PV matmul.
  - PV gives ctx^T which feeds o_proj as the stationary operand directly.
"""

import numpy as np

import concourse.bass as bass
import concourse.mybir as mybir
import concourse.tile as tile
from concourse import bacc
from concourse.bass_utils import run_bass_kernel_spmd

F32 = mybir.dt.float32
F32R = mybir.dt.float32r
AF = mybir.ActivationFunctionType
ALU = mybir.AluOpType

B, S, HID = 2, 2048, 2048
NH_TOT, NKV, D = 32, 4, 64
QD = 512          # q dims per core (8 heads x 64)
NH = 8            # q heads per core
CH = 512          # s-chunk width
NCH = S // CH     # 4
CT = HID // 128   # 16 contraction tiles
TT = S // 128     # 16 key tiles
P = 128

TRACE = False
LAST_EXEC_NS = None
LAST_RESULTS = None
_cache = {}


def _build(diag_mode):
    """diag_mode: 'causal' (skip masked t-tiles, affine_select diagonals),
    'none' (zero mask: dense, no masking), 'full' (arbitrary additive mask:
    dense + multiply every tile by host-provided exp(mask^T))."""
    nc = bacc.Bacc("TRN2", target_bir_lowering=False, debug=False, num_devices=8)

    x = nc.dram_tensor("x", [S, HID], F32R, kind="ExternalInput")
    wq = nc.dram_tensor("wq", [HID, QD], F32R, kind="ExternalInput")
    wkd = nc.dram_tensor("wkd", [HID, 128], F32R, kind="ExternalInput")
    wv = nc.dram_tensor("wv", [HID, D], F32R, kind="ExternalInput")
    wo = nc.dram_tensor("wo", [QD, HID], F32R, kind="ExternalInput")
    cosE = nc.dram_tensor("cosE", [P, S], F32R, kind="ExternalInput")
    ssE = nc.dram_tensor("ssE", [P, S], F32R, kind="ExternalInput")
    ident = nc.dram_tensor("ident", [P, P], F32R, kind="ExternalInput")
    cones = nc.dram_tensor("cones", [P, TT * D], F32R, kind="ExternalInput")
    m01 = None
    if diag_mode == "full":
        m01 = nc.dram_tensor("m01", [S, S], F32R, kind="ExternalInput")
    out = nc.dram_tensor("out", [S, HID], F32, kind="ExternalOutput")

    def n_ttiles(j):
        return 4 * j + 4 if diag_mode == "causal" else TT

    from contextlib import ExitStack
    with tile.TileContext(nc) as tc:
        with ExitStack() as ctx:
            pool = lambda n, b, space=None: ctx.enter_context(
                tc.tile_pool(name=n, bufs=b, **({"space": space} if space else {})))
            consts = pool("consts", 1)
            xnat_pool = pool("xnat", 2)
            xt_pool = pool("xt", 1)
            cs_pool = pool("cs", 2)
            wo_pool = pool("wos", 2)
            qT_pool = pool("qT", 5)
            rope_pool = pool("rope", 2)
            p_pool = pool("pp", 3)
            ctx_pool = pool("ctx", 5)
            octx_pool = pool("octx", 1)
            bc_pool = pool("bcs", 2)
            rcp_pool = pool("rcp", 2)
            out_pool = pool("outs", 3)
            m01_pool = pool("m01p", 4)
            ps_proj = pool("ps_proj", 2, "PSUM")
            ps_score = pool("ps_score", 2, "PSUM")
            ps_u = pool("ps_u", 2, "PSUM")

            # ---- resident constants ----
            wq_sb = consts.tile([P, CT, QD], F32R)
            wq_r = wq.ap().rearrange("(ct p) k -> p ct k", p=P)
            for qd in range(4):
                nc.sync.dma_start(out=wq_sb[:, 4 * qd:4 * (qd + 1), :],
                                  in_=wq_r[:, 4 * qd:4 * (qd + 1), :])
            wkd_sb = consts.tile([P, CT, 128], F32R)
            nc.sync.dma_start(out=wkd_sb, in_=wkd.ap().rearrange("(ct p) k -> p ct k", p=P))
            wv_sb = consts.tile([P, CT, D], F32R)
            nc.sync.dma_start(out=wv_sb, in_=wv.ap().rearrange("(ct p) k -> p ct k", p=P))
            id_sb = consts.tile([P, P], F32R)
            nc.sync.dma_start(out=id_sb, in_=ident.ap())
            kT_sb = consts.tile([P, S], F32R)        # [dup'd kv dim 128, t]
            # [t in tile, t-tile, V | ones64 | V] so either head parity gets a
            # contiguous [V|ones] / [ones|V] stationary slice
            v_aug = consts.tile([P, TT, 3 * D], F32R)
            nc.sync.dma_start(
                out=v_aug[:, :, D:2 * D],
                in_=cones.ap().rearrange("p (t d) -> p t d", t=TT))

            x_r = x.ap().rearrange("(ss p) c -> p ss c", p=P)  # ss: 16 blocks of 128 rows

            ctx_tiles = {}
            for j in range(NCH):
                jsl = slice(CH * j, CH * (j + 1))
                # ================= phase A: x^T, K/V/Q projections =================
                cosj = cs_pool.tile([P, CH], F32R, tag="cosj")
                nc.sync.dma_start(out=cosj, in_=cosE.ap()[:, jsl])
                ssj = cs_pool.tile([P, CH], F32R, tag="ssj")
                nc.sync.dma_start(out=ssj, in_=ssE.ap()[:, jsl])
                xt = xt_pool.tile([P, CT, CH], F32R, tag="xt")
                for sp in range(2):
                    xn0 = xnat_pool.tile([P, HID], F32R, tag="xn", name=f"xn_{j}_{sp}_0")
                    nc.sync.dma_start(out=xn0, in_=x_r[:, 4 * j + 2 * sp, :])
                    xn1 = xnat_pool.tile([P, HID], F32R, tag="xn", name=f"xn_{j}_{sp}_1")
                    nc.sync.dma_start(out=xn1, in_=x_r[:, 4 * j + 2 * sp + 1, :])
                    for ct in range(CT):
                        csl = slice(128 * ct, 128 * (ct + 1))
                        pt = ps_proj.tile([P, 256], F32R, tag="proj")
                        nc.tensor.transpose(pt[:, 0:128], xn0[:, csl], id_sb[:])
                        nc.tensor.transpose(pt[:, 128:256], xn1[:, csl], id_sb[:])
                        dst = xt[:, ct, 256 * sp:256 * (sp + 1)]
                        if ct % 2 == 0:
                            nc.vector.tensor_copy(out=dst, in_=pt)
                        else:
                            nc.scalar.copy(out=dst, in_=pt)

                # K projection (duplicated kv head) + rope -> kT_sb[:, j*CH:...]
                kp = ps_proj.tile([P, CH], F32, tag="proj")
                for ct in range(CT):
                    nc.tensor.matmul(
                        kp[:], wkd_sb[:, ct, :], xt[:, ct, :],
                        start=(ct == 0), stop=(ct == CT - 1),
                    )
                kraw = rope_pool.tile([P, CH], F32R, tag="rraw")
                nc.scalar.copy(out=kraw, in_=kp)
                kswp = rope_pool.tile([P, CH], F32R, tag="rswp")
                for half in range(4):
                    so = 32 * (half ^ 1)
                    nc.sync.dma_start(out=kswp[32 * half:32 * half + 32, :],
                                      in_=kraw[so:so + 32, :])
                kc = rope_pool.tile([P, CH], F32R, tag="rt1")
                nc.vector.tensor_mul(out=kc, in0=kraw, in1=cosj)
                ks = rope_pool.tile([P, CH], F32R, tag="rt2")
                nc.vector.tensor_mul(out=ks, in0=kswp, in1=ssj)
                nc.vector.tensor_add(out=kT_sb[:, jsl], in0=kc, in1=ks)

                # V projection -> natural layout (t on partitions) in v_aug
                vp = ps_proj.tile([P, CH], F32, tag="proj")
                for ct in range(CT):
                    nc.tensor.matmul(
                        vp[0:D, :], wv_sb[:, ct, :], xt[:, ct, :],
                        start=(ct == 0), stop=(ct == CT - 1),
                    )
                vT = rope_pool.tile([P, CH], F32R, tag="rraw")
                nc.scalar.copy(out=vT[0:D, :], in_=vp[0:D, :])
                for tl in range(4):
                    pv = ps_proj.tile([P, P], F32R, tag="proj")
                    nc.tensor.transpose(pv[:, 0:D], vT[0:D, 128 * tl:128 * (tl + 1)], id_sb[0:D, 0:D])
                    nc.vector.tensor_copy(out=v_aug[:, 4 * j + tl, 0:D], in_=pv[:, 0:D])
                    nc.vector.tensor_copy(out=v_aug[:, 4 * j + tl, 2 * D:3 * D], in_=pv[:, 0:D])

                # Q projection + rope -> qT tiles for this chunk
                qts = []
                for qt in range(4):
                    qp = ps_proj.tile([P, CH], F32, tag="proj")
                    for ct in range(CT):
                        nc.tensor.matmul(
                            qp[:], wq_sb[:, ct, 128 * qt:128 * (qt + 1)], xt[:, ct, :],
                            start=(ct == 0), stop=(ct == CT - 1),
                        )
                    qraw = rope_pool.tile([P, CH], F32R, tag="rraw")
                    nc.scalar.copy(out=qraw, in_=qp)
                    qswp = rope_pool.tile([P, CH], F32R, tag="rswp")
                    for half in range(4):
                        so = 32 * (half ^ 1)
                        nc.sync.dma_start(out=qswp[32 * half:32 * half + 32, :],
                                          in_=qraw[so:so + 32, :])
                    qc = rope_pool.tile([P, CH], F32R, tag="rt1")
                    nc.vector.tensor_mul(out=qc, in0=qraw, in1=cosj)
                    qs = rope_pool.tile([P, CH], F32R, tag="rt2")
                    nc.vector.tensor_mul(out=qs, in0=qswp, in1=ssj)
                    qT = qT_pool.tile([P, CH], F32R, tag="qT")
                    nc.vector.tensor_add(out=qT, in0=qc, in1=qs)
                    qts.append(qT)

                # ================= phase B: attention for s-chunk j =================
                nt = n_ttiles(j)
                for pair in range(4):
                    qt = pair
                    u_tiles = []
                    for par in range(2):
                        ut = ps_u.tile([P, CH], F32, tag="u", name=f"u_{j}_{pair}_{par}")
                        u_tiles.append(ut)
                    for k0 in range(0, nt, 2):
                        for par in range(2):
                            hp = 64 * par
                            q_ap = qts[qt][hp:hp + D, :]
                            u_ps = u_tiles[par]
                            sc = ps_score.tile([P, 2 * CH], F32, tag="score",
                                               name=f"sc_{j}_{pair}_{k0}_{par}")
                            for dk in range(2):
                                k = k0 + dk
                                nc.tensor.matmul(
                                    sc[:, CH * dk:CH * (dk + 1)],
                                    kT_sb[hp:hp + D, 128 * k:128 * (k + 1)], q_ap,
                                    start=True, stop=True,
                                )
                            ptile = p_pool.tile([P, 2 * CH], F32R, tag="p",
                                                name=f"p_{j}_{pair}_{k0}_{par}")
                            nc.scalar.activation(out=ptile, in_=sc, func=AF.Exp, scale=0.125)
                            for dk in range(2):
                                k = k0 + dk
                                psl = slice(CH * dk, CH * (dk + 1))
                                if diag_mode == "causal" and k >= 4 * j:
                                    nc.gpsimd.affine_select(
                                        out=ptile[:, psl], in_=ptile[:, psl],
                                        pattern=[[1, CH]],
                                        compare_op=ALU.is_ge, fill=0.0,
                                        base=CH * j - 128 * k, channel_multiplier=-1,
                                    )
                                elif diag_mode == "full":
                                    mt = m01_pool.tile([P, CH], F32R, tag="m01",
                                                       name=f"m_{j}_{pair}_{k0}_{par}_{dk}")
                                    nc.sync.dma_start(
                                        out=mt, in_=m01.ap()[128 * k:128 * (k + 1), jsl])
                                    nc.vector.tensor_mul(
                                        out=ptile[:, psl], in0=ptile[:, psl], in1=mt)
                                nc.tensor.matmul(
                                    u_ps[:], v_aug[:, k, hp:hp + 2 * D], ptile[:, psl],
                                    start=(k == 0), stop=(k == nt - 1),
                                )
                    for par in range(2):
                        u_ps = u_tiles[par]
                        rsl = slice(D, P) if par == 0 else slice(0, D)
                        csl_ = slice(0, D) if par == 0 else slice(D, P)
                        rcp = rcp_pool.tile([P, CH], F32, tag="rcp")
                        nc.vector.reciprocal(out=rcp[rsl, :], in_=u_ps[rsl, :])
                        rcp2 = rcp_pool.tile([P, CH], F32, tag="rcp2")
                        nc.sync.dma_start(out=rcp2[csl_, :], in_=rcp[rsl, :])
                        key = (j, qt)
                        if key not in ctx_tiles:
                            ctx_tiles[key] = ctx_pool.tile(
                                [P, CH], F32R, tag="ctx", name=f"ctx_{j}_{qt}")
                        ctx_t = ctx_tiles[key]
                        nc.vector.tensor_mul(
                            out=ctx_t[csl_, :], in0=u_ps[csl_, :], in1=rcp2[csl_, :])

                # ================= phase C: o_proj for s-chunk j =================
                for nck in range(4):
                    wo_nck = wo_pool.tile([P, 4, CH], F32R, tag="wo")
                    nc.sync.dma_start(
                        out=wo_nck,
                        in_=wo.ap()[:, CH * nck:CH * (nck + 1)].rearrange(
                            "(qt p) h -> p qt h", p=P))
                    for mi in range(4):
                        msl = slice(128 * mi, 128 * (mi + 1))
                        op = ps_proj.tile([P, CH], F32, tag="proj")
                        for qt in range(4):
                            nc.tensor.matmul(
                                op[:], ctx_tiles[(j, qt)][:, msl],
                                wo_nck[:, qt, :],
                                start=(qt == 0), stop=(qt == 3),
                            )
                        ot = out_pool.tile([P, CH], F32, tag="out")
                        nc.vector.tensor_copy(out=ot, in_=op)
                        nc.sync.dma_start(
                            out=out.ap()[CH * j + 128 * mi:CH * j + 128 * (mi + 1),
                                         CH * nck:CH * (nck + 1)],
                            in_=ot)

    nc.compile()
    return nc


def _classify_mask(mask):
    if not np.any(mask):
        return "none"
    tri = np.tril(np.ones(mask.shape, dtype=bool))
    if np.all(mask[tri] == 0.0) and np.all(mask[~tri] <= -1e8):
        return "causal"
    return "full"


def _host_inputs(x, cos, sin, mask, Wq, Wk, Wv, Wo, pos, diag_mode):
    pos = int(pos)
    perm = np.concatenate([np.arange(0, D, 2), np.arange(1, D, 2)])  # de-interleave
    cos_s = np.asarray(cos)[pos:pos + S].T.astype(np.float32)  # (32, S)
    sin_s = np.asarray(sin)[pos:pos + S].T.astype(np.float32)
    cosE = np.tile(np.concatenate([cos_s, cos_s], 0), (2, 1))       # (128, S)
    ssE = np.tile(np.concatenate([-sin_s, sin_s], 0), (2, 1))       # (128, S)
    ident = np.eye(P, dtype=np.float32)
    cones = np.ones((P, TT * D), dtype=np.float32)
    m01 = None
    if diag_mode == "full":
        m = np.asarray(mask, dtype=np.float64)
        m = m - m.max(axis=-1, keepdims=True)
        m01 = np.ascontiguousarray(np.exp(m).T.astype(np.float32))

    in_maps = []
    for c in range(8):
        b, g = divmod(c, 4)
        wq_c = np.asarray(Wq)[:, QD * g:QD * (g + 1)].reshape(HID, NH, D)[:, :, perm]
        wq_c = np.ascontiguousarray(wq_c.reshape(HID, QD), dtype=np.float32)
        wk_c = np.asarray(Wk)[:, D * g:D * (g + 1)][:, perm]
        wkd_c = np.ascontiguousarray(
            np.concatenate([wk_c, wk_c], axis=1), dtype=np.float32)
        wv_c = np.ascontiguousarray(np.asarray(Wv)[:, D * g:D * (g + 1)], dtype=np.float32)
        wo_c = np.ascontiguousarray(np.asarray(Wo)[QD * g:QD * (g + 1), :], dtype=np.float32)
        im = {
            "x": np.ascontiguousarray(np.asarray(x)[b], dtype=np.float32),
            "wq": wq_c, "wkd": wkd_c, "wv": wv_c, "wo": wo_c,
            "cosE": np.ascontiguousarray(cosE), "ssE": np.ascontiguousarray(ssE),
            "ident": ident, "cones": cones,
        }
        if m01 is not None:
            im["m01"] = m01
        in_maps.append(im)
    return in_maps


def kernel(x, cos, sin, mask, Wq, Wk, Wv, Wo, pos):
    global LAST_EXEC_NS, LAST_RESULTS
    diag_mode = _classify_mask(np.asarray(mask))
    if diag_mode not in _cache:
        _cache[diag_mode] = _build(diag_mode)
    nc = _cache[diag_mode]
    in_maps = _host_inputs(x, cos, sin, mask, Wq, Wk, Wv, Wo, pos, diag_mode)
    res = run_bass_kernel_spmd(nc, in_maps, core_ids=list(range(8)), trace=TRACE)
    LAST_EXEC_NS = res.exec_time_ns
    LAST_RESULTS = res
    full = np.zeros((B, S, HID), dtype=np.float32)
    for c in range(8):
        full[c // 4] += res.results[c]["out"]
    return full


# revision 19
# speedup vs baseline: 1.0128x; 1.0128x over previous
"""GQA attention (B=2, S=2048, 32 q heads / 4 kv heads, d=64, hidden=2048)
distributed over 8 TRN2 NeuronCores.

Sharding: core c = (batch b = c//4, kv-group g = c%4). Each core computes the
8 query heads of its kv-group for its batch, plus the partial o_proj for its
head slice; the host sums the 4 partial outputs per batch.

Layout strategy (per core):
  - x^T tiles (hidden on partitions) produced by TensorE transposes.
  - Q^T/K^T/V^T projections straight from x^T (all matmuls in float32r:
    fp32 with 12-bit mantissa rounding, 1 cycle/row on the PE).
  - RoPE applied in "de-interleaved" head layout (host permutes Wq/Wk
    columns so pair-swap = contiguous 32-partition block swap done by DMA).
  - scores^T (t on partitions, s on free) so softmax needs no transposes:
    exp on ScalarE (no max subtraction: scores are O(10), fp32 range is
    plenty), causal masking via affine_select on diagonal tiles only,
    row-sums come free as an extra ones-column in the PV matmul.
  - PV gives ctx^T which feeds o_proj as the stationary operand directly.
"""

import numpy as np

import concourse.bass as bass
import concourse.mybir as mybir
import concourse.tile as tile
from concourse import bacc
from concourse.bass_utils import run_bass_kernel_spmd

F32 = mybir.dt.float32
F32R = mybir.dt.float32r
AF = mybir.ActivationFunctionType
ALU = mybir.AluOpType

B, S, HID = 2, 2048, 2048
NH_TOT, NKV, D = 32, 4, 64
QD = 512          # q dims per core (8 heads x 64)
NH = 8            # q heads per core
CH = 512          # s-chunk width
NCH = S // CH     # 4
CT = HID // 128   # 16 contraction tiles
TT = S // 128     # 16 key tiles
P = 128

TRACE = False
LAST_EXEC_NS = None
LAST_RESULTS = None
_cache = {}


def _build(diag_mode):
    """diag_mode: 'causal' (skip masked t-tiles, affine_select diagonals),
    'none' (zero mask: dense, no masking), 'full' (arbitrary additive mask:
    dense + multiply every tile by host-provided exp(mask^T))."""
    nc = bacc.Bacc("TRN2", target_bir_lowering=False, debug=False, num_devices=8)

    x = nc.dram_tensor("x", [S, HID], F32R, kind="ExternalInput")
    wq = nc.dram_tensor("wq", [HID, QD], F32R, kind="ExternalInput")
    wkd = nc.dram_tensor("wkd", [HID, 128], F32R, kind="ExternalInput")
    wv = nc.dram_tensor("wv", [HID, D], F32R, kind="ExternalInput")
    wo = nc.dram_tensor("wo", [QD, HID], F32R, kind="ExternalInput")
    cosE = nc.dram_tensor("cosE", [P, S], F32R, kind="ExternalInput")
    ssE = nc.dram_tensor("ssE", [P, S], F32R, kind="ExternalInput")
    ident = nc.dram_tensor("ident", [P, P], F32R, kind="ExternalInput")
    cones = nc.dram_tensor("cones", [P, TT * D], F32R, kind="ExternalInput")
    m01 = None
    if diag_mode == "full":
        m01 = nc.dram_tensor("m01", [S, S], F32R, kind="ExternalInput")
    out = nc.dram_tensor("out", [S, HID], F32, kind="ExternalOutput")

    def n_ttiles(j):
        return 4 * j + 4 if diag_mode == "causal" else TT

    from contextlib import ExitStack
    with tile.TileContext(nc) as tc:
        with ExitStack() as ctx:
            pool = lambda n, b, space=None: ctx.enter_context(
                tc.tile_pool(name=n, bufs=b, **({"space": space} if space else {})))
            consts = pool("consts", 1)
            xnat_pool = pool("xnat", 2)
            xt_pool = pool("xt", 1)
            cs_pool = pool("cs", 2)
            wo_pool = pool("wos", 2)
            qT_pool = pool("qT", 5)
            rope_pool = pool("rope", 2)
            p_pool = pool("pp", 3)
            ctx_pool = pool("ctx", 5)
            octx_pool = pool("octx", 1)
            bc_pool = pool("bcs", 2)
            rcp_pool = pool("rcp", 2)
            out_pool = pool("outs", 3)
            m01_pool = pool("m01p", 4)
            ps_proj = pool("ps_proj", 2, "PSUM")
            ps_score = pool("ps_score", 2, "PSUM")
            ps_u = pool("ps_u", 2, "PSUM")

            # ---- resident constants ----
            wq_sb = consts.tile([P, CT, QD], F32R)
            wq_r = wq.ap().rearrange("(ct p) k -> p ct k", p=P)
            for qd in range(4):
                nc.sync.dma_start(out=wq_sb[:, 4 * qd:4 * (qd + 1), :],
                                  in_=wq_r[:, 4 * qd:4 * (qd + 1), :])
            wkd_sb = consts.tile([P, CT, 128], F32R)
            nc.sync.dma_start(out=wkd_sb, in_=wkd.ap().rearrange("(ct p) k -> p ct k", p=P))
            wv_sb = consts.tile([P, CT, D], F32R)
            nc.sync.dma_start(out=wv_sb, in_=wv.ap().rearrange("(ct p) k -> p ct k", p=P))
            id_sb = consts.tile([P, P], F32R)
            nc.sync.dma_start(out=id_sb, in_=ident.ap())
            kT_sb = consts.tile([P, S], F32R)        # [dup'd kv dim 128, t]
            # [t in tile, t-tile, V | ones64 | V] so either head parity gets a
            # contiguous [V|ones] / [ones|V] stationary slice
            v_aug = consts.tile([P, TT, 3 * D], F32R)
            nc.sync.dma_start(
                out=v_aug[:, :, D:2 * D],
                in_=cones.ap().rearrange("p (t d) -> p t d", t=TT))

            x_r = x.ap().rearrange("(ss p) c -> p ss c", p=P)  # ss: 16 blocks of 128 rows

            ctx_tiles = {}
            for j in range(NCH):
                jsl = slice(CH * j, CH * (j + 1))
                # ================= phase A: x^T, K/V/Q projections =================
                cosj = cs_pool.tile([P, CH], F32R, tag="cosj")
                nc.sync.dma_start(out=cosj, in_=cosE.ap()[:, jsl])
                ssj = cs_pool.tile([P, CH], F32R, tag="ssj")
                nc.sync.dma_start(out=ssj, in_=ssE.ap()[:, jsl])
                xt = xt_pool.tile([P, CT, CH], F32R, tag="xt")
                for sp in range(2):
                    xn0 = xnat_pool.tile([P, HID], F32R, tag="xn", name=f"xn_{j}_{sp}_0")
                    nc.sync.dma_start(out=xn0, in_=x_r[:, 4 * j + 2 * sp, :])
                    xn1 = xnat_pool.tile([P, HID], F32R, tag="xn", name=f"xn_{j}_{sp}_1")
                    nc.sync.dma_start(out=xn1, in_=x_r[:, 4 * j + 2 * sp + 1, :])
                    for ct in range(CT):
                        csl = slice(128 * ct, 128 * (ct + 1))
                        pt = ps_proj.tile([P, 256], F32R, tag="proj")
                        nc.tensor.transpose(pt[:, 0:128], xn0[:, csl], id_sb[:])
                        nc.tensor.transpose(pt[:, 128:256], xn1[:, csl], id_sb[:])
                        dst = xt[:, ct, 256 * sp:256 * (sp + 1)]
                        if ct % 2 == 0:
                            nc.vector.tensor_copy(out=dst, in_=pt)
                        else:
                            nc.scalar.copy(out=dst, in_=pt)

                # K projection (duplicated kv head) + rope -> kT_sb[:, j*CH:...]
                kp = ps_proj.tile([P, CH], F32, tag="proj")
                for ct in range(CT):
                    nc.tensor.matmul(
                        kp[:], wkd_sb[:, ct, :], xt[:, ct, :],
                        start=(ct == 0), stop=(ct == CT - 1),
                    )
                kraw = rope_pool.tile([P, CH], F32R, tag="rraw")
                nc.scalar.copy(out=kraw, in_=kp)
                kswp = rope_pool.tile([P, CH], F32R, tag="rswp")
                for half in range(4):
                    so = 32 * (half ^ 1)
                    nc.sync.dma_start(out=kswp[32 * half:32 * half + 32, :],
                                      in_=kraw[so:so + 32, :])
                kc = rope_pool.tile([P, CH], F32R, tag="rt1")
                nc.vector.tensor_mul(out=kc, in0=kraw, in1=cosj)
                ks = rope_pool.tile([P, CH], F32R, tag="rt2")
                nc.vector.tensor_mul(out=ks, in0=kswp, in1=ssj)
                nc.vector.tensor_add(out=kT_sb[:, jsl], in0=kc, in1=ks)

                # V projection -> natural layout (t on partitions) in v_aug
                vp = ps_proj.tile([P, CH], F32, tag="proj")
                for ct in range(CT):
                    nc.tensor.matmul(
                        vp[0:D, :], wv_sb[:, ct, :], xt[:, ct, :],
                        start=(ct == 0), stop=(ct == CT - 1),
                    )
                vT = rope_pool.tile([P, CH], F32R, tag="rraw")
                nc.scalar.copy(out=vT[0:D, :], in_=vp[0:D, :])
                for tl in range(4):
                    pv = ps_proj.tile([P, P], F32R, tag="proj")
                    nc.tensor.transpose(pv[:, 0:D], vT[0:D, 128 * tl:128 * (tl + 1)], id_sb[0:D, 0:D])
                    nc.vector.tensor_copy(out=v_aug[:, 4 * j + tl, 0:D], in_=pv[:, 0:D])
                    nc.vector.tensor_copy(out=v_aug[:, 4 * j + tl, 2 * D:3 * D], in_=pv[:, 0:D])

                # Q projection + rope -> qT tiles for this chunk
                qts = []
                for qt in range(4):
                    qp = ps_proj.tile([P, CH], F32, tag="proj")
                    for ct in range(CT):
                        nc.tensor.matmul(
                            qp[:], wq_sb[:, ct, 128 * qt:128 * (qt + 1)], xt[:, ct, :],
                            start=(ct == 0), stop=(ct == CT - 1),
                        )
                    qraw = rope_pool.tile([P, CH], F32R, tag="rraw")
                    nc.scalar.copy(out=qraw, in_=qp)
                    qswp = rope_pool.tile([P, CH], F32R, tag="rswp")
                    for half in range(4):
                        so = 32 * (half ^ 1)
                        nc.sync.dma_start(out=qswp[32 * half:32 * half + 32, :],
                                          in_=qraw[so:so + 32, :])
                    qc = rope_pool.tile([P, CH], F32R, tag="rt1")
                    nc.vector.tensor_mul(out=qc, in0=qraw, in1=cosj)
                    qs = rope_pool.tile([P, CH], F32R, tag="rt2")
                    nc.vector.tensor_mul(out=qs, in0=qswp, in1=ssj)
                    qT = qT_pool.tile([P, CH], F32R, tag="qT")
                    nc.vector.tensor_add(out=qT, in0=qc, in1=qs)
                    qts.append(qT)

                # ================= phase B: attention for s-chunk j =================
                nt = n_ttiles(j)
                for h in range(NH):
                    qt, hp = h // 2, 64 * (h % 2)
                    q_ap = qts[qt][hp:hp + D, :]
                    u_ps = ps_u.tile([P, CH], F32, tag="u", name=f"u_{j}_{h}")
                    for k0 in range(0, nt, 2):
                        sc = ps_score.tile([P, 2 * CH], F32, tag="score",
                                           name=f"sc_{j}_{h}_{k0}")
                        for dk in range(2):
                            k = k0 + dk
                            nc.tensor.matmul(
                                sc[:, CH * dk:CH * (dk + 1)],
                                kT_sb[hp:hp + D, 128 * k:128 * (k + 1)], q_ap,
                                start=True, stop=True,
                            )
                        ptile = p_pool.tile([P, 2 * CH], F32R, tag="p",
                                            name=f"p_{j}_{h}_{k0}")
                        nc.scalar.activation(out=ptile, in_=sc, func=AF.Exp, scale=0.125)
                        for dk in range(2):
                            k = k0 + dk
                            psl = slice(CH * dk, CH * (dk + 1))
                            if diag_mode == "causal" and k >= 4 * j:
                                nc.gpsimd.affine_select(
                                    out=ptile[:, psl], in_=ptile[:, psl],
                                    pattern=[[1, CH]],
                                    compare_op=ALU.is_ge, fill=0.0,
                                    base=CH * j - 128 * k, channel_multiplier=-1,
                                )
                            elif diag_mode == "full":
                                mt = m01_pool.tile([P, CH], F32R, tag="m01",
                                                   name=f"m_{j}_{h}_{k0}_{dk}")
                                nc.sync.dma_start(
                                    out=mt, in_=m01.ap()[128 * k:128 * (k + 1), jsl])
                                nc.vector.tensor_mul(
                                    out=ptile[:, psl], in0=ptile[:, psl], in1=mt)
                            nc.tensor.matmul(
                                u_ps[:], v_aug[:, k, hp:hp + 2 * D], ptile[:, psl],
                                start=(k == 0), stop=(k == nt - 1),
                            )
                    # normalize: ctx rows csl_, rowsum replicated in rows rsl
                    rsl = slice(D, P) if h % 2 == 0 else slice(0, D)
                    csl_ = slice(0, D) if h % 2 == 0 else slice(D, P)
                    rcp = rcp_pool.tile([P, CH], F32, tag="rcp")
                    nc.vector.reciprocal(out=rcp[rsl, :], in_=u_ps[rsl, :])
                    rcp2 = rcp_pool.tile([P, CH], F32, tag="rcp2")
                    nc.sync.dma_start(out=rcp2[csl_, :], in_=rcp[rsl, :])
                    key = (j, qt)
                    if key not in ctx_tiles:
                        ctx_tiles[key] = ctx_pool.tile(
                            [P, CH], F32R, tag="ctx", name=f"ctx_{j}_{qt}")
                    ctx_t = ctx_tiles[key]
                    nc.vector.tensor_mul(
                        out=ctx_t[csl_, :], in0=u_ps[csl_, :], in1=rcp2[csl_, :])

                # ================= phase C: o_proj for s-chunk j =================
                for nck in range(4):
                    wo_nck = wo_pool.tile([P, 4, CH], F32R, tag="wo")
                    nc.sync.dma_start(
                        out=wo_nck,
                        in_=wo.ap()[:, CH * nck:CH * (nck + 1)].rearrange(
                            "(qt p) h -> p qt h", p=P))
                    for mi in range(4):
                        msl = slice(128 * mi, 128 * (mi + 1))
                        op = ps_proj.tile([P, CH], F32, tag="proj")
                        for qt in range(4):
                            nc.tensor.matmul(
                                op[:], ctx_tiles[(j, qt)][:, msl],
                                wo_nck[:, qt, :],
                                start=(qt == 0), stop=(qt == 3),
                            )
                        ot = out_pool.tile([P, CH], F32, tag="out")
                        nc.vector.tensor_copy(out=ot, in_=op)
                        nc.sync.dma_start(
                            out=out.ap()[CH * j + 128 * mi:CH * j + 128 * (mi + 1),
                                         CH * nck:CH * (nck + 1)],
                            in_=ot)

    nc.compile()
    return nc


def _classify_mask(mask):
    if not np.any(mask):
        return "none"
    tri = np.tril(np.ones(mask.shape, dtype=bool))
    if np.all(mask[tri] == 0.0) and np.all(mask[~tri] <= -1e8):
        return "causal"
    return "full"


def _host_inputs(x, cos, sin, mask, Wq, Wk, Wv, Wo, pos, diag_mode):
    pos = int(pos)
    perm = np.concatenate([np.arange(0, D, 2), np.arange(1, D, 2)])  # de-interleave
    cos_s = np.asarray(cos)[pos:pos + S].T.astype(np.float32)  # (32, S)
    sin_s = np.asarray(sin)[pos:pos + S].T.astype(np.float32)
    cosE = np.tile(np.concatenate([cos_s, cos_s], 0), (2, 1))       # (128, S)
    ssE = np.tile(np.concatenate([-sin_s, sin_s], 0), (2, 1))       # (128, S)
    ident = np.eye(P, dtype=np.float32)
    cones = np.ones((P, TT * D), dtype=np.float32)
    m01 = None
    if diag_mode == "full":
        m = np.asarray(mask, dtype=np.float64)
        m = m - m.max(axis=-1, keepdims=True)
        m01 = np.ascontiguousarray(np.exp(m).T.astype(np.float32))

    in_maps = []
    for c in range(8):
        b, g = divmod(c, 4)
        wq_c = np.asarray(Wq)[:, QD * g:QD * (g + 1)].reshape(HID, NH, D)[:, :, perm]
        wq_c = np.ascontiguousarray(wq_c.reshape(HID, QD), dtype=np.float32)
        wk_c = np.asarray(Wk)[:, D * g:D * (g + 1)][:, perm]
        wkd_c = np.ascontiguousarray(
            np.concatenate([wk_c, wk_c], axis=1), dtype=np.float32)
        wv_c = np.ascontiguousarray(np.asarray(Wv)[:, D * g:D * (g + 1)], dtype=np.float32)
        wo_c = np.ascontiguousarray(np.asarray(Wo)[QD * g:QD * (g + 1), :], dtype=np.float32)
        im = {
            "x": np.ascontiguousarray(np.asarray(x)[b], dtype=np.float32),
            "wq": wq_c, "wkd": wkd_c, "wv": wv_c, "wo": wo_c,
            "cosE": np.ascontiguousarray(cosE), "ssE": np.ascontiguousarray(ssE),
            "ident": ident, "cones": cones,
        }
        if m01 is not None:
            im["m01"] = m01
        in_maps.append(im)
    return in_maps


def kernel(x, cos, sin, mask, Wq, Wk, Wv, Wo, pos):
    global LAST_EXEC_NS, LAST_RESULTS
    diag_mode = _classify_mask(np.asarray(mask))
    if diag_mode not in _cache:
        _cache[diag_mode] = _build(diag_mode)
    nc = _cache[diag_mode]
    in_maps = _host_inputs(x, cos, sin, mask, Wq, Wk, Wv, Wo, pos, diag_mode)
    res = run_bass_kernel_spmd(nc, in_maps, core_ids=list(range(8)), trace=TRACE)
    LAST_EXEC_NS = res.exec_time_ns
    LAST_RESULTS = res
    full = np.zeros((B, S, HID), dtype=np.float32)
    for c in range(8):
        full[c // 4] += res.results[c]["out"]
    return full


# revision 20
# speedup vs baseline: 1.2195x; 1.2042x over previous
"""GQA attention (B=2, S=2048, 32 q heads / 4 kv heads, d=64, hidden=2048)
distributed over 8 TRN2 NeuronCores.

Sharding: core c = (batch b = c//4, kv-group g = c%4). Each core computes the
8 query heads of its kv-group for its batch, plus the partial o_proj for its
head slice; the host sums the 4 partial outputs per batch.

Layout strategy (per core):
  - x^T tiles (hidden on partitions) produced by TensorE transposes.
  - Q^T/K^T/V^T projections straight from x^T (all matmuls in float32r:
    fp32 with 12-bit mantissa rounding, 1 cycle/row on the PE).
  - RoPE applied in "de-interleaved" head layout (host permutes Wq/Wk
    columns so pair-swap = contiguous 32-partition block swap done by DMA).
  - scores^T (t on partitions, s on free) so softmax needs no transposes:
    exp on ScalarE (no max subtraction: scores are O(10), fp32 range is
    plenty), causal masking via affine_select on diagonal tiles only,
    row-sums come free as an extra ones-column in the PV matmul.
  - PV gives ctx^T which feeds o_proj as the stationary operand directly.
"""

import numpy as np

import concourse.bass as bass
import concourse.mybir as mybir
import concourse.tile as tile
from concourse import bacc
from concourse.bass_utils import run_bass_kernel_spmd

F32 = mybir.dt.float32
F32R = mybir.dt.float32r
AF = mybir.ActivationFunctionType
ALU = mybir.AluOpType

B, S, HID = 2, 2048, 2048
NH_TOT, NKV, D = 32, 4, 64
QD = 512          # q dims per core (8 heads x 64)
NH = 8            # q heads per core
CH = 512          # s-chunk width
NCH = S // CH     # 4
CT = HID // 128   # 16 contraction tiles
TT = S // 128     # 16 key tiles
P = 128

TRACE = False
LAST_EXEC_NS = None
LAST_RESULTS = None
_cache = {}


def _build(diag_mode):
    """diag_mode: 'causal' (skip masked t-tiles, affine_select diagonals),
    'none' (zero mask: dense, no masking), 'full' (arbitrary additive mask:
    dense + multiply every tile by host-provided exp(mask^T))."""
    nc = bacc.Bacc("TRN2", target_bir_lowering=False, debug=False, num_devices=8)

    x = nc.dram_tensor("x", [S, HID], F32R, kind="ExternalInput")
    wq = nc.dram_tensor("wq", [HID, QD], F32R, kind="ExternalInput")
    wkd = nc.dram_tensor("wkd", [HID, 128], F32R, kind="ExternalInput")
    wv = nc.dram_tensor("wv", [HID, D], F32R, kind="ExternalInput")
    wo = nc.dram_tensor("wo", [QD, HID], F32R, kind="ExternalInput")
    cosE = nc.dram_tensor("cosE", [P, S], F32R, kind="ExternalInput")
    ssE = nc.dram_tensor("ssE", [P, S], F32R, kind="ExternalInput")
    ident = nc.dram_tensor("ident", [P, P], F32R, kind="ExternalInput")
    cones = nc.dram_tensor("cones", [P, TT * D], F32R, kind="ExternalInput")
    m01 = None
    if diag_mode == "full":
        m01 = nc.dram_tensor("m01", [S, S], F32R, kind="ExternalInput")
    out = nc.dram_tensor("out", [S, HID], F32, kind="ExternalOutput")

    def n_ttiles(j):
        return 4 * j + 4 if diag_mode == "causal" else TT

    from contextlib import ExitStack
    with tile.TileContext(nc) as tc:
        with ExitStack() as ctx:
            pool = lambda n, b, space=None: ctx.enter_context(
                tc.tile_pool(name=n, bufs=b, **({"space": space} if space else {})))
            consts = pool("consts", 1)
            xnat_pool = pool("xnat", 2)
            xt_pool = pool("xt", 1)
            cs_pool = pool("cs", 2)
            wo_pool = pool("wos", 2)
            qT_pool = pool("qT", 5)
            rope_pool = pool("rope", 2)
            p_pool = pool("pp", 3)
            ctx_pool = pool("ctx", 5)
            octx_pool = pool("octx", 1)
            bc_pool = pool("bcs", 2)
            rcp_pool = pool("rcp", 2)
            usb_pool = pool("usb", 2)
            out_pool = pool("outs", 3)
            m01_pool = pool("m01p", 4)
            ps_proj = pool("ps_proj", 2, "PSUM")
            ps_score = pool("ps_score", 2, "PSUM")
            ps_u = pool("ps_u", 1, "PSUM")
            ps_o = pool("ps_o", 1, "PSUM")

            # ---- resident constants ----
            wq_sb = consts.tile([P, CT, QD], F32R)
            wq_r = wq.ap().rearrange("(ct p) k -> p ct k", p=P)
            for qd in range(4):
                nc.sync.dma_start(out=wq_sb[:, 4 * qd:4 * (qd + 1), :],
                                  in_=wq_r[:, 4 * qd:4 * (qd + 1), :])
            wkd_sb = consts.tile([P, CT, 128], F32R)
            nc.sync.dma_start(out=wkd_sb, in_=wkd.ap().rearrange("(ct p) k -> p ct k", p=P))
            wv_sb = consts.tile([P, CT, D], F32R)
            nc.sync.dma_start(out=wv_sb, in_=wv.ap().rearrange("(ct p) k -> p ct k", p=P))
            id_sb = consts.tile([P, P], F32R)
            nc.sync.dma_start(out=id_sb, in_=ident.ap())
            kT_sb = consts.tile([P, S], F32R)        # [dup'd kv dim 128, t]
            # [t in tile, t-tile, V | ones64 | V] so either head parity gets a
            # contiguous [V|ones] / [ones|V] stationary slice
            v_aug = consts.tile([P, TT, 3 * D], F32R)
            nc.sync.dma_start(
                out=v_aug[:, :, D:2 * D],
                in_=cones.ap().rearrange("p (t d) -> p t d", t=TT))

            x_r = x.ap().rearrange("(ss p) c -> p ss c", p=P)  # ss: 16 blocks of 128 rows

            ctx_tiles = {}
            for j in range(NCH):
                jsl = slice(CH * j, CH * (j + 1))
                # ================= phase A: x^T, K/V/Q projections =================
                cosj = cs_pool.tile([P, CH], F32R, tag="cosj")
                nc.sync.dma_start(out=cosj, in_=cosE.ap()[:, jsl])
                ssj = cs_pool.tile([P, CH], F32R, tag="ssj")
                nc.sync.dma_start(out=ssj, in_=ssE.ap()[:, jsl])
                xt = xt_pool.tile([P, CT, CH], F32R, tag="xt")
                for sp in range(2):
                    xn0 = xnat_pool.tile([P, HID], F32R, tag="xn", name=f"xn_{j}_{sp}_0")
                    nc.sync.dma_start(out=xn0, in_=x_r[:, 4 * j + 2 * sp, :])
                    xn1 = xnat_pool.tile([P, HID], F32R, tag="xn", name=f"xn_{j}_{sp}_1")
                    nc.sync.dma_start(out=xn1, in_=x_r[:, 4 * j + 2 * sp + 1, :])
                    for ct in range(CT):
                        csl = slice(128 * ct, 128 * (ct + 1))
                        pt = ps_proj.tile([P, 256], F32R, tag="proj")
                        nc.tensor.transpose(pt[:, 0:128], xn0[:, csl], id_sb[:])
                        nc.tensor.transpose(pt[:, 128:256], xn1[:, csl], id_sb[:])
                        dst = xt[:, ct, 256 * sp:256 * (sp + 1)]
                        if ct % 2 == 0:
                            nc.vector.tensor_copy(out=dst, in_=pt)
                        else:
                            nc.scalar.copy(out=dst, in_=pt)

                # K projection (duplicated kv head) + rope -> kT_sb[:, j*CH:...]
                kp = ps_proj.tile([P, CH], F32, tag="proj")
                for ct in range(CT):
                    nc.tensor.matmul(
                        kp[:], wkd_sb[:, ct, :], xt[:, ct, :],
                        start=(ct == 0), stop=(ct == CT - 1),
                    )
                kraw = rope_pool.tile([P, CH], F32R, tag="rraw")
                nc.scalar.copy(out=kraw, in_=kp)
                kswp = rope_pool.tile([P, CH], F32R, tag="rswp")
                for half in range(4):
                    so = 32 * (half ^ 1)
                    nc.sync.dma_start(out=kswp[32 * half:32 * half + 32, :],
                                      in_=kraw[so:so + 32, :])
                kc = rope_pool.tile([P, CH], F32R, tag="rt1")
                nc.vector.tensor_mul(out=kc, in0=kraw, in1=cosj)
                ks = rope_pool.tile([P, CH], F32R, tag="rt2")
                nc.vector.tensor_mul(out=ks, in0=kswp, in1=ssj)
                nc.vector.tensor_add(out=kT_sb[:, jsl], in0=kc, in1=ks)

                # V projection -> natural layout (t on partitions) in v_aug
                vp = ps_proj.tile([P, CH], F32, tag="proj")
                for ct in range(CT):
                    nc.tensor.matmul(
                        vp[0:D, :], wv_sb[:, ct, :], xt[:, ct, :],
                        start=(ct == 0), stop=(ct == CT - 1),
                    )
                vT = rope_pool.tile([P, CH], F32R, tag="rraw")
                nc.scalar.copy(out=vT[0:D, :], in_=vp[0:D, :])
                for tl in range(4):
                    pv = ps_proj.tile([P, P], F32R, tag="proj")
                    nc.tensor.transpose(pv[:, 0:D], vT[0:D, 128 * tl:128 * (tl + 1)], id_sb[0:D, 0:D])
                    nc.vector.tensor_copy(out=v_aug[:, 4 * j + tl, 0:D], in_=pv[:, 0:D])
                    nc.vector.tensor_copy(out=v_aug[:, 4 * j + tl, 2 * D:3 * D], in_=pv[:, 0:D])

                # Q projection + rope -> qT tiles for this chunk
                qts = []
                for qt in range(4):
                    qp = ps_proj.tile([P, CH], F32, tag="proj")
                    for ct in range(CT):
                        nc.tensor.matmul(
                            qp[:], wq_sb[:, ct, 128 * qt:128 * (qt + 1)], xt[:, ct, :],
                            start=(ct == 0), stop=(ct == CT - 1),
                        )
                    qraw = rope_pool.tile([P, CH], F32R, tag="rraw")
                    nc.scalar.copy(out=qraw, in_=qp)
                    qswp = rope_pool.tile([P, CH], F32R, tag="rswp")
                    for half in range(4):
                        so = 32 * (half ^ 1)
                        nc.sync.dma_start(out=qswp[32 * half:32 * half + 32, :],
                                          in_=qraw[so:so + 32, :])
                    qc = rope_pool.tile([P, CH], F32R, tag="rt1")
                    nc.vector.tensor_mul(out=qc, in0=qraw, in1=cosj)
                    qs = rope_pool.tile([P, CH], F32R, tag="rt2")
                    nc.vector.tensor_mul(out=qs, in0=qswp, in1=ssj)
                    qT = qT_pool.tile([P, CH], F32R, tag="qT")
                    nc.vector.tensor_add(out=qT, in0=qc, in1=qs)
                    qts.append(qT)

                # ================= phase B: attention for s-chunk j =================
                nt = n_ttiles(j)
                for h in range(NH):
                    qt, hp = h // 2, 64 * (h % 2)
                    q_ap = qts[qt][hp:hp + D, :]
                    u_ps = ps_u.tile([P, CH], F32, tag="u", name=f"u_{j}_{h}")
                    for k0 in range(0, nt, 2):
                        sc = ps_score.tile([P, 2 * CH], F32, tag="score",
                                           name=f"sc_{j}_{h}_{k0}")
                        for dk in range(2):
                            k = k0 + dk
                            nc.tensor.matmul(
                                sc[:, CH * dk:CH * (dk + 1)],
                                kT_sb[hp:hp + D, 128 * k:128 * (k + 1)], q_ap,
                                start=True, stop=True,
                            )
                        ptile = p_pool.tile([P, 2 * CH], F32R, tag="p",
                                            name=f"p_{j}_{h}_{k0}")
                        nc.scalar.activation(out=ptile, in_=sc, func=AF.Exp, scale=0.125)
                        for dk in range(2):
                            k = k0 + dk
                            psl = slice(CH * dk, CH * (dk + 1))
                            if diag_mode == "causal" and k >= 4 * j:
                                nc.gpsimd.affine_select(
                                    out=ptile[:, psl], in_=ptile[:, psl],
                                    pattern=[[1, CH]],
                                    compare_op=ALU.is_ge, fill=0.0,
                                    base=CH * j - 128 * k, channel_multiplier=-1,
                                )
                            elif diag_mode == "full":
                                mt = m01_pool.tile([P, CH], F32R, tag="m01",
                                                   name=f"m_{j}_{h}_{k0}_{dk}")
                                nc.sync.dma_start(
                                    out=mt, in_=m01.ap()[128 * k:128 * (k + 1), jsl])
                                nc.vector.tensor_mul(
                                    out=ptile[:, psl], in0=ptile[:, psl], in1=mt)
                            nc.tensor.matmul(
                                u_ps[:], v_aug[:, k, hp:hp + 2 * D], ptile[:, psl],
                                start=(k == 0), stop=(k == nt - 1),
                            )
                    # evac U fast (frees the PSUM slot for the next head),
                    # then normalize off-PSUM: ctx rows csl_, rowsum rows rsl
                    usb = usb_pool.tile([P, CH], F32, tag="usb")
                    nc.scalar.copy(out=usb, in_=u_ps)
                    rsl = slice(D, P) if h % 2 == 0 else slice(0, D)
                    csl_ = slice(0, D) if h % 2 == 0 else slice(D, P)
                    rcp = rcp_pool.tile([P, CH], F32, tag="rcp")
                    nc.vector.reciprocal(out=rcp[rsl, :], in_=usb[rsl, :])
                    rcp2 = rcp_pool.tile([P, CH], F32, tag="rcp2")
                    nc.sync.dma_start(out=rcp2[csl_, :], in_=rcp[rsl, :])
                    key = (j, qt)
                    if key not in ctx_tiles:
                        ctx_tiles[key] = ctx_pool.tile(
                            [P, CH], F32R, tag="ctx", name=f"ctx_{j}_{qt}")
                    ctx_t = ctx_tiles[key]
                    nc.vector.tensor_mul(
                        out=ctx_t[csl_, :], in0=usb[csl_, :], in1=rcp2[csl_, :])

                # ================= phase C: o_proj for s-chunk j =================
                for nck in range(4):
                    wo_nck = wo_pool.tile([P, 4, CH], F32R, tag="wo")
                    nc.sync.dma_start(
                        out=wo_nck,
                        in_=wo.ap()[:, CH * nck:CH * (nck + 1)].rearrange(
                            "(qt p) h -> p qt h", p=P))
                    for mi in range(4):
                        msl = slice(128 * mi, 128 * (mi + 1))
                        op = ps_o.tile([P, CH], F32, tag="opsum")
                        for qt in range(4):
                            nc.tensor.matmul(
                                op[:], ctx_tiles[(j, qt)][:, msl],
                                wo_nck[:, qt, :],
                                start=(qt == 0), stop=(qt == 3),
                            )
                        ot = out_pool.tile([P, CH], F32, tag="out")
                        nc.vector.tensor_copy(out=ot, in_=op)
                        nc.sync.dma_start(
                            out=out.ap()[CH * j + 128 * mi:CH * j + 128 * (mi + 1),
                                         CH * nck:CH * (nck + 1)],
                            in_=ot)

    nc.compile()
    return nc


def _classify_mask(mask):
    if not np.any(mask):
        return "none"
    tri = np.tril(np.ones(mask.shape, dtype=bool))
    if np.all(mask[tri] == 0.0) and np.all(mask[~tri] <= -1e8):
        return "causal"
    return "full"


def _host_inputs(x, cos, sin, mask, Wq, Wk, Wv, Wo, pos, diag_mode):
    pos = int(pos)
    perm = np.concatenate([np.arange(0, D, 2), np.arange(1, D, 2)])  # de-interleave
    cos_s = np.asarray(cos)[pos:pos + S].T.astype(np.float32)  # (32, S)
    sin_s = np.asarray(sin)[pos:pos + S].T.astype(np.float32)
    cosE = np.tile(np.concatenate([cos_s, cos_s], 0), (2, 1))       # (128, S)
    ssE = np.tile(np.concatenate([-sin_s, sin_s], 0), (2, 1))       # (128, S)
    ident = np.eye(P, dtype=np.float32)
    cones = np.ones((P, TT * D), dtype=np.float32)
    m01 = None
    if diag_mode == "full":
        m = np.asarray(mask, dtype=np.float64)
        m = m - m.max(axis=-1, keepdims=True)
        m01 = np.ascontiguousarray(np.exp(m).T.astype(np.float32))

    in_maps = []
    for c in range(8):
        b, g = divmod(c, 4)
        wq_c = np.asarray(Wq)[:, QD * g:QD * (g + 1)].reshape(HID, NH, D)[:, :, perm]
        wq_c = np.ascontiguousarray(wq_c.reshape(HID, QD), dtype=np.float32)
        wk_c = np.asarray(Wk)[:, D * g:D * (g + 1)][:, perm]
        wkd_c = np.ascontiguousarray(
            np.concatenate([wk_c, wk_c], axis=1), dtype=np.float32)
        wv_c = np.ascontiguousarray(np.asarray(Wv)[:, D * g:D * (g + 1)], dtype=np.float32)
        wo_c = np.ascontiguousarray(np.asarray(Wo)[QD * g:QD * (g + 1), :], dtype=np.float32)
        im = {
            "x": np.ascontiguousarray(np.asarray(x)[b], dtype=np.float32),
            "wq": wq_c, "wkd": wkd_c, "wv": wv_c, "wo": wo_c,
            "cosE": np.ascontiguousarray(cosE), "ssE": np.ascontiguousarray(ssE),
            "ident": ident, "cones": cones,
        }
        if m01 is not None:
            im["m01"] = m01
        in_maps.append(im)
    return in_maps


def kernel(x, cos, sin, mask, Wq, Wk, Wv, Wo, pos):
    global LAST_EXEC_NS, LAST_RESULTS
    diag_mode = _classify_mask(np.asarray(mask))
    if diag_mode not in _cache:
        _cache[diag_mode] = _build(diag_mode)
    nc = _cache[diag_mode]
    in_maps = _host_inputs(x, cos, sin, mask, Wq, Wk, Wv, Wo, pos, diag_mode)
    res = run_bass_kernel_spmd(nc, in_maps, core_ids=list(range(8)), trace=TRACE)
    LAST_EXEC_NS = res.exec_time_ns
    LAST_RESULTS = res
    full = np.zeros((B, S, HID), dtype=np.float32)
    for c in range(8):
        full[c // 4] += res.results[c]["out"]
    return full


# revision 22
# speedup vs baseline: 1.2335x; 1.0114x over previous
"""GQA attention (B=2, S=2048, 32 q heads / 4 kv heads, d=64, hidden=2048)
distributed over 8 TRN2 NeuronCores.

Sharding: core c = (batch b = c//4, kv-group g = c%4). Each core computes the
8 query heads of its kv-group for its batch, plus the partial o_proj for its
head slice; the host sums the 4 partial outputs per batch.

Layout strategy (per core):
  - x^T tiles (hidden on partitions) produced by TensorE transposes.
  - Q^T/K^T/V^T projections straight from x^T (all matmuls in float32r:
    fp32 with 12-bit mantissa rounding, 1 cycle/row on the PE).
  - RoPE applied in "de-interleaved" head layout (host permutes Wq/Wk
    columns so pair-swap = contiguous 32-partition block swap done by DMA).
  - scores^T (t on partitions, s on free) so softmax needs no transposes:
    exp on ScalarE (no max subtraction: scores are O(10), fp32 range is
    plenty), causal masking via affine_select on diagonal tiles only,
    row-sums come free as an extra ones-column in the PV matmul.
  - PV gives ctx^T which feeds o_proj as the stationary operand directly.
"""

import numpy as np

import concourse.bass as bass
import concourse.mybir as mybir
import concourse.tile as tile
from concourse import bacc
from concourse.bass_utils import run_bass_kernel_spmd

F32 = mybir.dt.float32
F32R = mybir.dt.float32r
AF = mybir.ActivationFunctionType
ALU = mybir.AluOpType

B, S, HID = 2, 2048, 2048
NH_TOT, NKV, D = 32, 4, 64
QD = 512          # q dims per core (8 heads x 64)
NH = 8            # q heads per core
CH = 512          # s-chunk width
NCH = S // CH     # 4
CT = HID // 128   # 16 contraction tiles
TT = S // 128     # 16 key tiles
P = 128

TRACE = False
LAST_EXEC_NS = None
LAST_RESULTS = None
_cache = {}


def _build(diag_mode):
    """diag_mode: 'causal' (skip masked t-tiles, affine_select diagonals),
    'none' (zero mask: dense, no masking), 'full' (arbitrary additive mask:
    dense + multiply every tile by host-provided exp(mask^T))."""
    nc = bacc.Bacc("TRN2", target_bir_lowering=False, debug=False, num_devices=8)

    x = nc.dram_tensor("x", [S, HID], F32R, kind="ExternalInput")
    wq = nc.dram_tensor("wq", [HID, QD], F32R, kind="ExternalInput")
    wkd = nc.dram_tensor("wkd", [HID, 128], F32R, kind="ExternalInput")
    wv = nc.dram_tensor("wv", [HID, D], F32R, kind="ExternalInput")
    wo = nc.dram_tensor("wo", [QD, HID], F32R, kind="ExternalInput")
    cosE = nc.dram_tensor("cosE", [P, S], F32R, kind="ExternalInput")
    ssE = nc.dram_tensor("ssE", [P, S], F32R, kind="ExternalInput")
    ident = nc.dram_tensor("ident", [P, P], F32R, kind="ExternalInput")
    cones = nc.dram_tensor("cones", [P, TT * D], F32R, kind="ExternalInput")
    m01 = None
    if diag_mode == "full":
        m01 = nc.dram_tensor("m01", [S, S], F32R, kind="ExternalInput")
    out = nc.dram_tensor("out", [S, HID], F32, kind="ExternalOutput")

    def n_ttiles(j):
        return 4 * j + 4 if diag_mode == "causal" else TT

    from contextlib import ExitStack
    with tile.TileContext(nc) as tc:
        with ExitStack() as ctx:
            pool = lambda n, b, space=None: ctx.enter_context(
                tc.tile_pool(name=n, bufs=b, **({"space": space} if space else {})))
            consts = pool("consts", 1)
            xnat_pool = pool("xnat", 2)
            xt_pool = pool("xt", 1)
            cs_pool = pool("cs", 2)
            wo_pool = pool("wos", 2)
            qT_pool = pool("qT", 5)
            rope_pool = pool("rope", 2)
            p_pool = pool("pp", 3)
            ctx_pool = pool("ctx", 5)
            octx_pool = pool("octx", 1)
            bc_pool = pool("bcs", 2)
            rcp_pool = pool("rcp", 2)
            usb_pool = pool("usb", 2)
            out_pool = pool("outs", 3)
            m01_pool = pool("m01p", 4)
            ps_proj = pool("ps_proj", 2, "PSUM")
            ps_score = pool("ps_score", 2, "PSUM")
            ps_u = pool("ps_u", 1, "PSUM")
            ps_o = pool("ps_o", 1, "PSUM")

            # ---- resident constants (weight DMAs are emitted inside the
            # j==0 loop body so the first x chunk's DMAs go out first) ----
            wq_sb = consts.tile([P, CT, QD], F32R)
            wkd_sb = consts.tile([P, CT, 128], F32R)
            wv_sb = consts.tile([P, CT, D], F32R)
            id_sb = consts.tile([P, P], F32R)
            nc.sync.dma_start(out=id_sb, in_=ident.ap())
            kT_sb = consts.tile([P, S], F32R)        # [dup'd kv dim 128, t]
            # [t in tile, t-tile, V | ones64 | V] so either head parity gets a
            # contiguous [V|ones] / [ones|V] stationary slice
            v_aug = consts.tile([P, TT, 3 * D], F32R)

            x_r = x.ap().rearrange("(ss p) c -> p ss c", p=P)  # ss: 16 blocks of 128 rows

            ctx_tiles = {}
            for j in range(NCH):
                jsl = slice(CH * j, CH * (j + 1))
                # ================= phase A: x^T, K/V/Q projections =================
                cosj = cs_pool.tile([P, CH], F32R, tag="cosj")
                nc.sync.dma_start(out=cosj, in_=cosE.ap()[:, jsl])
                ssj = cs_pool.tile([P, CH], F32R, tag="ssj")
                nc.sync.dma_start(out=ssj, in_=ssE.ap()[:, jsl])
                xt = xt_pool.tile([P, CT, CH], F32R, tag="xt")
                for sp in range(2):
                    xn0 = xnat_pool.tile([P, HID], F32R, tag="xn", name=f"xn_{j}_{sp}_0")
                    nc.sync.dma_start(out=xn0, in_=x_r[:, 4 * j + 2 * sp, :])
                    xn1 = xnat_pool.tile([P, HID], F32R, tag="xn", name=f"xn_{j}_{sp}_1")
                    nc.sync.dma_start(out=xn1, in_=x_r[:, 4 * j + 2 * sp + 1, :])
                    if j == 0 and sp == 0:
                        nc.sync.dma_start(
                            out=wkd_sb, in_=wkd.ap().rearrange("(ct p) k -> p ct k", p=P))
                        nc.sync.dma_start(
                            out=wv_sb, in_=wv.ap().rearrange("(ct p) k -> p ct k", p=P))
                        wq_r = wq.ap().rearrange("(ct p) k -> p ct k", p=P)
                        for qd in range(4):
                            nc.sync.dma_start(out=wq_sb[:, 4 * qd:4 * (qd + 1), :],
                                              in_=wq_r[:, 4 * qd:4 * (qd + 1), :])
                        nc.sync.dma_start(
                            out=v_aug[:, :, D:2 * D],
                            in_=cones.ap().rearrange("p (t d) -> p t d", t=TT))
                    for ct in range(CT):
                        csl = slice(128 * ct, 128 * (ct + 1))
                        pt = ps_proj.tile([P, 256], F32R, tag="proj")
                        nc.tensor.transpose(pt[:, 0:128], xn0[:, csl], id_sb[:])
                        nc.tensor.transpose(pt[:, 128:256], xn1[:, csl], id_sb[:])
                        dst = xt[:, ct, 256 * sp:256 * (sp + 1)]
                        nc.vector.tensor_copy(out=dst, in_=pt)

                # K projection (duplicated kv head) + rope -> kT_sb[:, j*CH:...]
                kp = ps_proj.tile([P, CH], F32, tag="proj")
                for ct in range(CT):
                    nc.tensor.matmul(
                        kp[:], wkd_sb[:, ct, :], xt[:, ct, :],
                        start=(ct == 0), stop=(ct == CT - 1),
                    )
                kraw = rope_pool.tile([P, CH], F32R, tag="rraw")
                nc.vector.tensor_copy(out=kraw, in_=kp)
                kswp = rope_pool.tile([P, CH], F32R, tag="rswp")
                for half in range(4):
                    so = 32 * (half ^ 1)
                    nc.sync.dma_start(out=kswp[32 * half:32 * half + 32, :],
                                      in_=kraw[so:so + 32, :])
                kc = rope_pool.tile([P, CH], F32R, tag="rt1")
                nc.vector.tensor_mul(out=kc, in0=kraw, in1=cosj)
                ks = rope_pool.tile([P, CH], F32R, tag="rt2")
                nc.vector.tensor_mul(out=ks, in0=kswp, in1=ssj)
                nc.vector.tensor_add(out=kT_sb[:, jsl], in0=kc, in1=ks)

                # V projection -> natural layout (t on partitions) in v_aug
                vp = ps_proj.tile([P, CH], F32, tag="proj")
                for ct in range(CT):
                    nc.tensor.matmul(
                        vp[0:D, :], wv_sb[:, ct, :], xt[:, ct, :],
                        start=(ct == 0), stop=(ct == CT - 1),
                    )
                vT = rope_pool.tile([P, CH], F32R, tag="rraw")
                nc.vector.tensor_copy(out=vT[0:D, :], in_=vp[0:D, :])
                for tl in range(4):
                    pv = ps_proj.tile([P, P], F32R, tag="proj")
                    nc.tensor.transpose(pv[:, 0:D], vT[0:D, 128 * tl:128 * (tl + 1)], id_sb[0:D, 0:D])
                    nc.vector.tensor_copy(out=v_aug[:, 4 * j + tl, 0:D], in_=pv[:, 0:D])
                    nc.vector.tensor_copy(out=v_aug[:, 4 * j + tl, 2 * D:3 * D], in_=pv[:, 0:D])

                # Q projection + rope -> qT tiles for this chunk
                qts = []
                for qt in range(4):
                    qp = ps_proj.tile([P, CH], F32, tag="proj")
                    for ct in range(CT):
                        nc.tensor.matmul(
                            qp[:], wq_sb[:, ct, 128 * qt:128 * (qt + 1)], xt[:, ct, :],
                            start=(ct == 0), stop=(ct == CT - 1),
                        )
                    qraw = rope_pool.tile([P, CH], F32R, tag="rraw")
                    nc.vector.tensor_copy(out=qraw, in_=qp)
                    qswp = rope_pool.tile([P, CH], F32R, tag="rswp")
                    for half in range(4):
                        so = 32 * (half ^ 1)
                        nc.sync.dma_start(out=qswp[32 * half:32 * half + 32, :],
                                          in_=qraw[so:so + 32, :])
                    qc = rope_pool.tile([P, CH], F32R, tag="rt1")
                    nc.vector.tensor_mul(out=qc, in0=qraw, in1=cosj)
                    qs = rope_pool.tile([P, CH], F32R, tag="rt2")
                    nc.vector.tensor_mul(out=qs, in0=qswp, in1=ssj)
                    qT = qT_pool.tile([P, CH], F32R, tag="qT")
                    nc.vector.tensor_add(out=qT, in0=qc, in1=qs)
                    qts.append(qT)

                # ================= phase B: attention for s-chunk j =================
                nt = n_ttiles(j)
                for h in range(NH):
                    qt, hp = h // 2, 64 * (h % 2)
                    q_ap = qts[qt][hp:hp + D, :]
                    u_ps = ps_u.tile([P, CH], F32, tag="u", name=f"u_{j}_{h}")
                    for k0 in range(0, nt, 2):
                        sc = ps_score.tile([P, 2 * CH], F32, tag="score",
                                           name=f"sc_{j}_{h}_{k0}")
                        for dk in range(2):
                            k = k0 + dk
                            nc.tensor.matmul(
                                sc[:, CH * dk:CH * (dk + 1)],
                                kT_sb[hp:hp + D, 128 * k:128 * (k + 1)], q_ap,
                                start=True, stop=True,
                            )
                        ptile = p_pool.tile([P, 2 * CH], F32R, tag="p",
                                            name=f"p_{j}_{h}_{k0}")
                        nc.scalar.activation(out=ptile, in_=sc, func=AF.Exp, scale=0.125)
                        for dk in range(2):
                            k = k0 + dk
                            psl = slice(CH * dk, CH * (dk + 1))
                            if diag_mode == "causal" and k >= 4 * j:
                                nc.gpsimd.affine_select(
                                    out=ptile[:, psl], in_=ptile[:, psl],
                                    pattern=[[1, CH]],
                                    compare_op=ALU.is_ge, fill=0.0,
                                    base=CH * j - 128 * k, channel_multiplier=-1,
                                )
                            elif diag_mode == "full":
                                mt = m01_pool.tile([P, CH], F32R, tag="m01",
                                                   name=f"m_{j}_{h}_{k0}_{dk}")
                                nc.sync.dma_start(
                                    out=mt, in_=m01.ap()[128 * k:128 * (k + 1), jsl])
                                nc.vector.tensor_mul(
                                    out=ptile[:, psl], in0=ptile[:, psl], in1=mt)
                            nc.tensor.matmul(
                                u_ps[:], v_aug[:, k, hp:hp + 2 * D], ptile[:, psl],
                                start=(k == 0), stop=(k == nt - 1),
                            )
                    # evac U fast (frees the PSUM slot for the next head),
                    # then normalize off-PSUM: ctx rows csl_, rowsum rows rsl
                    usb = usb_pool.tile([P, CH], F32, tag="usb")
                    nc.vector.tensor_copy(out=usb, in_=u_ps)
                    rsl = slice(D, P) if h % 2 == 0 else slice(0, D)
                    csl_ = slice(0, D) if h % 2 == 0 else slice(D, P)
                    rcp = rcp_pool.tile([P, CH], F32, tag="rcp")
                    nc.vector.reciprocal(out=rcp[rsl, :], in_=usb[rsl, :])
                    rcp2 = rcp_pool.tile([P, CH], F32, tag="rcp2")
                    nc.sync.dma_start(out=rcp2[csl_, :], in_=rcp[rsl, :])
                    key = (j, qt)
                    if key not in ctx_tiles:
                        ctx_tiles[key] = ctx_pool.tile(
                            [P, CH], F32R, tag="ctx", name=f"ctx_{j}_{qt}")
                    ctx_t = ctx_tiles[key]
                    nc.vector.tensor_mul(
                        out=ctx_t[csl_, :], in0=usb[csl_, :], in1=rcp2[csl_, :])

                # ================= phase C: o_proj for s-chunk j =================
                for nck in range(4):
                    wo_nck = wo_pool.tile([P, 4, CH], F32R, tag="wo")
                    nc.sync.dma_start(
                        out=wo_nck,
                        in_=wo.ap()[:, CH * nck:CH * (nck + 1)].rearrange(
                            "(qt p) h -> p qt h", p=P))
                    for mi in range(4):
                        msl = slice(128 * mi, 128 * (mi + 1))
                        op = ps_o.tile([P, CH], F32, tag="opsum")
                        for qt in range(4):
                            nc.tensor.matmul(
                                op[:], ctx_tiles[(j, qt)][:, msl],
                                wo_nck[:, qt, :],
                                start=(qt == 0), stop=(qt == 3),
                            )
                        ot = out_pool.tile([P, CH], F32, tag="out")
                        nc.vector.tensor_copy(out=ot, in_=op)
                        nc.sync.dma_start(
                            out=out.ap()[CH * j + 128 * mi:CH * j + 128 * (mi + 1),
                                         CH * nck:CH * (nck + 1)],
                            in_=ot)

    nc.compile()
    return nc


def _classify_mask(mask):
    if not np.any(mask):
        return "none"
    tri = np.tril(np.ones(mask.shape, dtype=bool))
    if np.all(mask[tri] == 0.0) and np.all(mask[~tri] <= -1e8):
        return "causal"
    return "full"


def _host_inputs(x, cos, sin, mask, Wq, Wk, Wv, Wo, pos, diag_mode):
    pos = int(pos)
    perm = np.concatenate([np.arange(0, D, 2), np.arange(1, D, 2)])  # de-interleave
    cos_s = np.asarray(cos)[pos:pos + S].T.astype(np.float32)  # (32, S)
    sin_s = np.asarray(sin)[pos:pos + S].T.astype(np.float32)
    cosE = np.tile(np.concatenate([cos_s, cos_s], 0), (2, 1))       # (128, S)
    ssE = np.tile(np.concatenate([-sin_s, sin_s], 0), (2, 1))       # (128, S)
    ident = np.eye(P, dtype=np.float32)
    cones = np.ones((P, TT * D), dtype=np.float32)
    m01 = None
    if diag_mode == "full":
        m = np.asarray(mask, dtype=np.float64)
        m = m - m.max(axis=-1, keepdims=True)
        m01 = np.ascontiguousarray(np.exp(m).T.astype(np.float32))

    in_maps = []
    for c in range(8):
        b, g = divmod(c, 4)
        wq_c = np.asarray(Wq)[:, QD * g:QD * (g + 1)].reshape(HID, NH, D)[:, :, perm]
        wq_c = np.ascontiguousarray(wq_c.reshape(HID, QD), dtype=np.float32)
        wk_c = np.asarray(Wk)[:, D * g:D * (g + 1)][:, perm]
        wkd_c = np.ascontiguousarray(
            np.concatenate([wk_c, wk_c], axis=1), dtype=np.float32)
        wv_c = np.ascontiguousarray(np.asarray(Wv)[:, D * g:D * (g + 1)], dtype=np.float32)
        wo_c = np.ascontiguousarray(np.asarray(Wo)[QD * g:QD * (g + 1), :], dtype=np.float32)
        im = {
            "x": np.ascontiguousarray(np.asarray(x)[b], dtype=np.float32),
            "wq": wq_c, "wkd": wkd_c, "wv": wv_c, "wo": wo_c,
            "cosE": np.ascontiguousarray(cosE), "ssE": np.ascontiguousarray(ssE),
            "ident": ident, "cones": cones,
        }
        if m01 is not None:
            im["m01"] = m01
        in_maps.append(im)
    return in_maps


def kernel(x, cos, sin, mask, Wq, Wk, Wv, Wo, pos):
    global LAST_EXEC_NS, LAST_RESULTS
    diag_mode = _classify_mask(np.asarray(mask))
    if diag_mode not in _cache:
        _cache[diag_mode] = _build(diag_mode)
    nc = _cache[diag_mode]
    in_maps = _host_inputs(x, cos, sin, mask, Wq, Wk, Wv, Wo, pos, diag_mode)
    res = run_bass_kernel_spmd(nc, in_maps, core_ids=list(range(8)), trace=TRACE)
    LAST_EXEC_NS = res.exec_time_ns
    LAST_RESULTS = res
    full = np.zeros((B, S, HID), dtype=np.float32)
    for c in range(8):
        full[c // 4] += res.results[c]["out"]
    return full
